# revision 1
# baseline (speedup 1.0000x reference)
"""Dense CRF mean-field inference (2 labels) on 8 Trainium2 NeuronCores.

Strategy (column-sharded, collective-synchronized):
  - N = 80*80 = 6400 pixels. Core c owns the contiguous i-block
    I_c = [c*800, (c+1)*800).
  - Both pairwise kernels are Gaussians of squared feature distances, so
    K[j,i] = exp(f_j.f_i - |f_i|^2/2 - |f_j|^2/2) is built on the tensor
    engine via an augmented-feature gram matmul ([f_i; -|f_i|^2/2] against
    [f_j; 1]), with the per-j offset applied as the activation bias of the
    Exp that materializes each [128, 800] tile (fp16). The Gaussian kernel
    block (and the first KB_CACHE tiles of each bilateral block) stay
    resident in SBUF across iterations; the rest stream from HBM.
  - Row sums for the symmetric normalization fall out of the Exp's
    accum_out (free-axis sum = partial sum over the core's i-block);
    an AllReduce produces full row sums (j-layout) and a ReduceScatter on
    a rank-major reshuffle hands each core the sums for its own i-block.
  - Each mean-field step: out[l, i] += (Q * n_j)-stationary matmuls
    accumulated over all 50 j-tiles into PSUM; a PE transpose flips
    messages to [i-partition, l] layout for the epilogue (norm scaling,
    compat weights, unary add, 2-label softmax as a sigmoid). New Q rows
    are AllGathered (102 KB) for the next iteration.
  - Per-call device traffic is minimized: image/prediction-dependent data
    is a single [800, 16] tensor per core, broadcast to all cores by an
    AllGather at kernel start; everything else is image-independent and
    cached on-device across calls.
"""

import sys

if "/opt/trn_rl_repo" not in sys.path:
    sys.path.insert(0, "/opt/trn_rl_repo")

import numpy as np

import concourse.bass as bass
import concourse.tile as tile
from concourse import bacc, mybir

# ----- problem constants (hardcoded per the harness contract) -----
B, H, W = 2, 80, 80
N = H * W                 # 6400
P = 128                   # SBUF partitions
NT = N // P               # 50 j-tiles
N_CORES = 8
CHUNK = N // N_CORES      # 800 i's per core
NIC = (CHUNK + P - 1) // P  # 7 i-chunks (6x128 + 32)
I_CHUNKS = [(q * P, min(P, CHUNK - q * P)) for q in range(NIC)]
KB_CACHE = 22             # bilateral j-tiles kept SBUF-resident per image

SXY_G, COMPAT_G = 3.0, 3.0
SXY_B, SRGB_B, COMPAT_B = 50.0, 5.0, 10.0
CLIP = 1e-5

F32 = mybir.dt.float32
KDT = mybir.dt.float16    # K storage / matmul operand dtype

RG = [list(range(N_CORES))]

# fresh input column map: [CHUNK, 16]
#   0:3  img0 colors * 255/SRGB_B (r,g,b rows of my block)
#   3:6  img1 colors * 255/SRGB_B
#   6    -|f_b0|^2/2 for my block     7   -|f_b1|^2/2
#   8:12 Q_init (c = 2*img + l)      12:16 -U (same column order)
FRESH_COLS = 16

_RUNNER_CACHE: dict = {}


def build_program(iters: int):
    nc = bacc.Bacc(
        "TRN2", target_bir_lowering=False, debug=False, num_devices=N_CORES
    )

    def inp(name, shape):
        return nc.dram_tensor(name, list(shape), F32, kind="ExternalInput").ap()

    # static (image-independent) inputs — cached on device across calls
    Lgpos = inp("Lgpos", (3, N))      # rows: x/3, y/3, ones
    Lbpos = inp("Lbpos", (2, N))      # rows: x/50, y/50
    biasg = inp("biasg", (N,))        # -|f_g|^2/2
    Rgq = inp("Rgq", (3, CHUNK))      # x/3, y/3, -|f_g|^2/2  (my block)
    Rbpos = inp("Rbpos", (2, CHUNK))  # x/50, y/50            (my block)
    ident = inp("ident", (P, P))
    # per-call input
    fresh = inp("fresh", (CHUNK, FRESH_COLS))
    qout = nc.dram_tensor("qout", [CHUNK, 2], F32, kind="ExternalOutput").ap()

    AF = mybir.ActivationFunctionType
    OP = mybir.AluOpType

    with tile.TileContext(nc) as tc:
        with (
            tc.tile_pool(name="const", bufs=1) as cpool,
            tc.tile_pool(name="dram", bufs=1, space="DRAM") as dpool,
        ):
            # ---- broadcast the per-call data to all cores ----
            fresh_b = dpool.tile([CHUNK, FRESH_COLS], F32, tag="fresh_b")
            ag0_out = dpool.tile([N, FRESH_COLS], F32, tag="ag0_out")
            nc.sync.dma_start(fresh_b[:], fresh)
            nc.gpsimd.collective_compute(
                "AllGather",
                OP.bypass,
                replica_groups=RG,
                ins=[fresh_b.opt()],
                outs=[ag0_out.opt()],
            )

            # ---- persistent SBUF ----
            biasg_sb = cpool.tile([P, NT], F32)
            biasb0_sb = cpool.tile([P, NT], F32)
            biasb1_sb = cpool.tile([P, NT], F32)
            negU_sb = cpool.tile([P, NIC, 4], F32)
            ident_sb = cpool.tile([P, P], F32)
            ones4 = cpool.tile([P, 4], F32)
            rowsum_g = cpool.tile([P, NT], F32)
            rowsum_b0 = cpool.tile([P, NT], F32)
            rowsum_b1 = cpool.tile([P, NT], F32)
            # SBUF-resident kernel blocks
            kg_cache = cpool.tile([P, NT, CHUNK], KDT)
            kb0_cache = cpool.tile([P, KB_CACHE, CHUNK], KDT)
            kb1_cache = cpool.tile([P, KB_CACHE, CHUNK], KDT)

            nc.sync.dma_start(biasg_sb[:], biasg.rearrange("(t p) -> p t", p=P))
            nc.sync.dma_start(
                biasb0_sb[:], ag0_out[:, 6].rearrange("(t p) -> p t", p=P)
            )
            nc.sync.dma_start(
                biasb1_sb[:], ag0_out[:, 7].rearrange("(t p) -> p t", p=P)
            )
            nc.sync.dma_start(
                negU_sb[:, 0:6, :],
                fresh[0 : 6 * P, 12:16].rearrange("(q p) c -> p q c", p=P),
            )
            nc.sync.dma_start(negU_sb[0:32, 6, :], fresh[6 * P : CHUNK, 12:16])
            nc.sync.dma_start(ident_sb[:], ident)
            nc.vector.memset(ones4[:], 1.0)

            # streamed bilateral K storage in HBM (tiles >= KB_CACHE)
            KB0 = dpool.tile([NT, P, CHUNK], KDT, tag="KB0")
            KB1 = dpool.tile([NT, P, CHUNK], KDT, tag="KB1")

            # ---------- build phase ----------
            with (
                tc.tile_pool(name="bconst", bufs=1) as bpool,
                tc.tile_pool(name="bpsum", bufs=4, space="PSUM") as bpsum,
                tc.tile_pool(name="bk", bufs=4) as bkpool,
            ):
                # packed feature rows: g at base 0, b0 at 32, b1 at 64
                L_sb = bpool.tile([70, N], F32)
                R_sb = bpool.tile([70, CHUNK], F32)
                nc.sync.dma_start(L_sb[0:3, :], Lgpos)
                nc.sync.dma_start(L_sb[32:34, :], Lbpos)
                nc.sync.dma_start(
                    L_sb[34:37, :], ag0_out[:, 0:3].rearrange("n c -> c n")
                )
                nc.sync.dma_start(L_sb[37:38, :], Lgpos[2:3, :])
                nc.sync.dma_start(L_sb[64:66, :], Lbpos)
                nc.sync.dma_start(
                    L_sb[66:69, :], ag0_out[:, 3:6].rearrange("n c -> c n")
                )
                nc.sync.dma_start(L_sb[69:70, :], Lgpos[2:3, :])

                nc.sync.dma_start(R_sb[0:3, :], Rgq)
                nc.sync.dma_start(R_sb[32:34, :], Rbpos)
                nc.sync.dma_start(
                    R_sb[34:37, :], fresh[:, 0:3].rearrange("n c -> c n")
                )
                nc.sync.dma_start(
                    R_sb[37:38, :], fresh[:, 6:7].rearrange("n c -> c n")
                )
                nc.sync.dma_start(R_sb[64:66, :], Rbpos)
                nc.sync.dma_start(
                    R_sb[66:69, :], fresh[:, 3:6].rearrange("n c -> c n")
                )
                nc.sync.dma_start(
                    R_sb[69:70, :], fresh[:, 7:8].rearrange("n c -> c n")
                )

                for t in range(NT):
                    specs = [
                        ("g", 0, 3, biasg_sb, rowsum_g, kg_cache, None, None),
                        ("b0", 32, 38, biasb0_sb, rowsum_b0, kb0_cache, KB0,
                         None),
                        ("b1", 64, 70, biasb1_sb, rowsum_b1, kb1_cache, KB1,
                         None),
                    ]
                    for knm, r0, r1, bias_sb, rsum, cache, KD, _ in specs:
                        ps = bpsum.tile(
                            [P, CHUNK], F32, tag="ps", name=f"ps{knm}{t}"
                        )
                        nc.tensor.matmul(
                            ps[:, 0:512],
                            lhsT=L_sb[r0:r1, bass.ts(t, P)],
                            rhs=R_sb[r0:r1, 0:512],
                            start=True,
                            stop=True,
                        )
                        nc.tensor.matmul(
                            ps[:, 512:CHUNK],
                            lhsT=L_sb[r0:r1, bass.ts(t, P)],
                            rhs=R_sb[r0:r1, 512:CHUNK],
                            start=True,
                            stop=True,
                        )
                        in_cache = KD is None or t < KB_CACHE
                        if in_cache:
                            kdst = cache[:, t, :]
                        else:
                            kdst = bkpool.tile(
                                [P, CHUNK], KDT, tag="kt", name=f"kt{knm}{t}"
                            )[:]
                        nc.scalar.activation(
                            kdst,
                            ps[:],
                            AF.Exp,
                            bias=bias_sb[:, t : t + 1],
                            scale=1.0,
                            accum_out=rsum[:, t : t + 1],
                        )
                        if not in_cache:
                            nc.sync.dma_start(KD[t], kdst)

            # ---------- normalization collectives ----------
            rs_in = dpool.tile([3, N], F32, tag="rs_in")
            rs_out = dpool.tile([3, N], F32, tag="rs_out")
            rs2_in = dpool.tile([N_CORES, 3, CHUNK], F32, tag="rs2_in")
            rs2_out = dpool.tile([3, CHUNK], F32, tag="rs2_out")

            for k, rsum in enumerate((rowsum_g, rowsum_b0, rowsum_b1)):
                nc.sync.dma_start(
                    rs_in[k].rearrange("(t p) -> p t", p=P), rsum[:]
                )
            nc.sync.dma_start(
                rs2_in[:], rs_in.rearrange("k (r m) -> r k m", r=N_CORES)
            )
            nc.gpsimd.collective_compute(
                "AllReduce",
                OP.add,
                replica_groups=RG,
                ins=[rs_in.opt()],
                outs=[rs_out.opt()],
            )
            nc.gpsimd.collective_compute(
                "ReduceScatter",
                OP.add,
                replica_groups=RG,
                ins=[rs2_in.opt()],
                outs=[rs2_out.opt()],
            )

            # ---------- norms ----------
            sums_j = cpool.tile([P, 3 * NT], F32)
            for k in range(3):
                nc.sync.dma_start(
                    sums_j[:, k * NT : (k + 1) * NT],
                    rs_out[k].rearrange("(t p) -> p t", p=P),
                )
            recip_j = cpool.tile([P, 3 * NT], F32)
            nc.vector.reciprocal(recip_j[:], sums_j[:])
            normj = cpool.tile([P, 3 * NT], F32)
            nc.scalar.activation(normj[:], recip_j[:], AF.Sqrt)

            ngrep = cpool.tile([P, NT, 4], F32)
            nbrep = cpool.tile([P, NT, 4], F32)
            for t in range(NT):
                nc.vector.tensor_scalar(
                    ngrep[:, t, :], ones4[:], normj[:, t : t + 1], None, OP.mult
                )
                nc.vector.tensor_scalar(
                    nbrep[:, t, 0:2],
                    ones4[:, 0:2],
                    normj[:, NT + t : NT + t + 1],
                    None,
                    OP.mult,
                )
                nc.vector.tensor_scalar(
                    nbrep[:, t, 2:4],
                    ones4[:, 0:2],
                    normj[:, 2 * NT + t : 2 * NT + t + 1],
                    None,
                    OP.mult,
                )

            sums_i = cpool.tile([P, 3 * NIC], F32)
            nc.vector.memset(sums_i[:], 1.0)
            for k in range(3):
                nc.sync.dma_start(
                    sums_i[:, k * NIC : k * NIC + 6],
                    rs2_out[k, 0 : 6 * P].rearrange("(q p) -> p q", p=P),
                )
                nc.sync.dma_start(
                    sums_i[0:32, k * NIC + 6], rs2_out[k, 6 * P : CHUNK]
                )
            recip_i = cpool.tile([P, 3 * NIC], F32)
            nc.vector.reciprocal(recip_i[:], sums_i[:])
            normi = cpool.tile([P, 3 * NIC], F32)
            nc.scalar.activation(normi[:], recip_i[:], AF.Sqrt)

            # ---------- mean-field iterations ----------
            with (
                tc.tile_pool(name="kstream", bufs=6) as kpool,
                tc.tile_pool(name="acc", bufs=1, space="PSUM") as accpool,
                tc.tile_pool(name="tr", bufs=2, space="PSUM") as trpool,
                tc.tile_pool(name="ep", bufs=2) as eppool,
                tc.tile_pool(name="eps", bufs=8) as epspool,
                tc.tile_pool(name="q", bufs=2) as qpool,
            ):
                for it in range(iters):
                    last = it == iters - 1

                    q_all = qpool.tile(
                        [P, NT, 4], F32, tag="q_all", name=f"q_all{it}"
                    )
                    if it == 0:
                        nc.sync.dma_start(
                            q_all[:],
                            ag0_out[:, 8:12].rearrange("(t p) c -> p t c", p=P),
                        )
                    else:
                        nc.sync.dma_start(
                            q_all[:], qsrc.rearrange("(t p) c -> p t c", p=P)
                        )
                    rhs_g = qpool.tile(
                        [P, NT, 4], KDT, tag="rhs_g", name=f"rhs_g{it}"
                    )
                    rhs_b = qpool.tile(
                        [P, NT, 4], KDT, tag="rhs_b", name=f"rhs_b{it}"
                    )
                    nc.vector.tensor_tensor(rhs_g[:], q_all[:], ngrep[:], OP.mult)
                    nc.vector.tensor_tensor(rhs_b[:], q_all[:], nbrep[:], OP.mult)

                    pg = accpool.tile([4, CHUNK], F32, tag="pg", name=f"pg{it}")
                    pb0 = accpool.tile([2, CHUNK], F32, tag="pb0", name=f"pb0{it}")
                    pb1 = accpool.tile([2, CHUNK], F32, tag="pb1", name=f"pb1{it}")

                    for t in range(NT):
                        kg = kg_cache[:, t, :]
                        if t < KB_CACHE:
                            kb0 = kb0_cache[:, t, :]
                            kb1 = kb1_cache[:, t, :]
                        else:
                            kb0t = kpool.tile(
                                [P, CHUNK], KDT, tag="kb0", name=f"kb0_{it}_{t}"
                            )
                            kb1t = kpool.tile(
                                [P, CHUNK], KDT, tag="kb1", name=f"kb1_{it}_{t}"
                            )
                            nc.sync.dma_start(kb0t[:], KB0[t])
                            nc.sync.dma_start(kb1t[:], KB1[t])
                            kb0 = kb0t[:]
                            kb1 = kb1t[:]
                        st = dict(start=(t == 0), stop=(t == NT - 1))
                        for c0, cn in ((0, 512), (512, CHUNK - 512)):
                            nc.tensor.matmul(
                                pg[:, c0 : c0 + cn],
                                lhsT=rhs_g[:, t, :],
                                rhs=kg[:, c0 : c0 + cn],
                                **st,
                            )
                            nc.tensor.matmul(
                                pb0[:, c0 : c0 + cn],
                                lhsT=rhs_b[:, t, 0:2],
                                rhs=kb0[:, c0 : c0 + cn],
                                **st,
                            )
                            nc.tensor.matmul(
                                pb1[:, c0 : c0 + cn],
                                lhsT=rhs_b[:, t, 2:4],
                                rhs=kb1[:, c0 : c0 + cn],
                                **st,
                            )

                    # epilogue: PSUM -> SBUF, transpose to [i, l], softmax
                    sg = eppool.tile([4, CHUNK], F32, tag="sg", name=f"sg{it}")
                    sb0 = eppool.tile([2, CHUNK], F32, tag="sb0", name=f"sb0{it}")
                    sb1 = eppool.tile([2, CHUNK], F32, tag="sb1", name=f"sb1{it}")
                    nc.vector.tensor_copy(sg[:], pg[:])
                    nc.vector.tensor_copy(sb0[:], pb0[:])
                    nc.vector.tensor_copy(sb1[:], pb1[:])

                    qstage = qpool.tile(
                        [P, NIC, 4], F32, tag="qstage", name=f"qstage{it}"
                    )
                    for q, (i0, iw) in enumerate(I_CHUNKS):
                        tr = trpool.tile([P, 8], F32, tag="tr", name=f"tr{it}_{q}")
                        nc.tensor.transpose(
                            tr[0:iw, 0:4], sg[:, i0 : i0 + iw], ident_sb[0:4, 0:4]
                        )
                        nc.tensor.transpose(
                            tr[0:iw, 4:6], sb0[:, i0 : i0 + iw],
                            ident_sb[0:2, 0:2],
                        )
                        nc.tensor.transpose(
                            tr[0:iw, 6:8], sb1[:, i0 : i0 + iw],
                            ident_sb[0:2, 0:2],
                        )
                        ag = epspool.tile([P, 4], F32, tag="ag", name=f"ag{it}_{q}")
                        ab = epspool.tile([P, 4], F32, tag="ab", name=f"ab{it}_{q}")
                        nc.vector.tensor_scalar(
                            ag[0:iw, :],
                            tr[0:iw, 0:4],
                            normi[0:iw, q : q + 1],
                            COMPAT_G,
                            OP.mult,
                            OP.mult,
                        )
                        nc.vector.tensor_scalar(
                            ab[0:iw, 0:2],
                            tr[0:iw, 4:6],
                            normi[0:iw, NIC + q : NIC + q + 1],
                            COMPAT_B,
                            OP.mult,
                            OP.mult,
                        )
                        nc.vector.tensor_scalar(
                            ab[0:iw, 2:4],
                            tr[0:iw, 6:8],
                            normi[0:iw, 2 * NIC + q : 2 * NIC + q + 1],
                            COMPAT_B,
                            OP.mult,
                            OP.mult,
                        )
                        s = epspool.tile([P, 4], F32, tag="s", name=f"s{it}_{q}")
                        nc.vector.tensor_tensor(
                            s[0:iw, :], ag[0:iw, :], ab[0:iw, :], OP.add
                        )
                        nc.vector.tensor_tensor(
                            s[0:iw, :], s[0:iw, :], negU_sb[0:iw, q, :], OP.add
                        )
                        d = epspool.tile([P, 2], F32, tag="d", name=f"d{it}_{q}")
                        nc.vector.tensor_tensor(
                            d[0:iw, 0:1], s[0:iw, 0:1], s[0:iw, 1:2], OP.subtract
                        )
                        nc.vector.tensor_tensor(
                            d[0:iw, 1:2], s[0:iw, 2:3], s[0:iw, 3:4], OP.subtract
                        )
                        if last:
                            nc.scalar.activation(
                                qstage[0:iw, q, 0:1], d[0:iw, 0:1], AF.Sigmoid
                            )
                            nc.scalar.activation(
                                qstage[0:iw, q, 1:2], d[0:iw, 1:2], AF.Sigmoid
                            )
                        else:
                            nc.scalar.activation(
                                qstage[0:iw, q, 0:1], d[0:iw, 0:1], AF.Sigmoid
                            )
                            nc.scalar.activation(
                                qstage[0:iw, q, 1:2], d[0:iw, 0:1], AF.Sigmoid,
                                scale=-1.0,
                            )
                            nc.scalar.activation(
                                qstage[0:iw, q, 2:3], d[0:iw, 1:2], AF.Sigmoid
                            )
                            nc.scalar.activation(
                                qstage[0:iw, q, 3:4], d[0:iw, 1:2], AF.Sigmoid,
                                scale=-1.0,
                            )

                    if last:
                        nc.sync.dma_start(
                            qout[0 : 6 * P, :].rearrange("(q p) c -> p q c", p=P),
                            qstage[:, 0:6, 0:2],
                        )
                        nc.sync.dma_start(
                            qout[6 * P : CHUNK, :], qstage[0:32, 6, 0:2]
                        )
                    else:
                        qag_in = dpool.tile(
                            [CHUNK, 4], F32, tag=f"qag_in{it}", name=f"qag_in{it}"
                        )
                        qag_out = dpool.tile(
                            [N, 4], F32, tag=f"qag_out{it}", name=f"qag_out{it}"
                        )
                        nc.sync.dma_start(
                            qag_in[0 : 6 * P, :].rearrange("(q p) c -> p q c", p=P),
                            qstage[:, 0:6, :],
                        )
                        nc.sync.dma_start(
                            qag_in[6 * P : CHUNK, :], qstage[0:32, 6, :]
                        )
                        nc.gpsimd.collective_compute(
                            "AllGather",
                            OP.bypass,
                            replica_groups=RG,
                            ins=[qag_in.opt()],
                            outs=[qag_out.opt()],
                        )
                        qsrc = qag_out

    nc.compile()
    return nc


# ---------------- host-side data ----------------

def static_inputs():
    """Image-independent per-core inputs (device-cacheable)."""
    yy, xx = np.mgrid[0:H, 0:W]
    pos = np.stack([xx.ravel(), yy.ravel()], 1).astype(np.float64)  # [N,2]
    fg = pos / SXY_G
    q2g = 0.5 * (fg * fg).sum(1)
    fbpos = pos / SXY_B

    f32 = lambda a: np.ascontiguousarray(a, dtype=np.float32)
    shared = {
        "Lgpos": f32(np.concatenate([fg.T, np.ones((1, N))], 0)),
        "Lbpos": f32(fbpos.T),
        "biasg": f32(-q2g),
        "ident": np.eye(P, dtype=np.float32),
    }
    maps = []
    for c in range(N_CORES):
        sl = slice(c * CHUNK, (c + 1) * CHUNK)
        m = dict(shared)
        m["Rgq"] = f32(np.concatenate([fg[sl].T, -q2g[None, sl]], 0))
        m["Rbpos"] = f32(fbpos[sl].T)
        maps.append(m)
    return maps


def fresh_inputs(img: np.ndarray, pred: np.ndarray):
    """Per-call per-core [CHUNK, 16] tensors + host Q_init (for iters=0)."""
    yy, xx = np.mgrid[0:H, 0:W]
    pos = np.stack([xx.ravel(), yy.ravel()], 1).astype(np.float64)
    colors = img.reshape(B, 3, N).transpose(0, 2, 1).astype(np.float64) * 255.0
    cb = colors / SRGB_B                               # [B,N,3]
    fbpos = pos / SXY_B
    q2b = [
        0.5 * ((fbpos * fbpos).sum(1) + (cb[b] * cb[b]).sum(1)) for b in range(B)
    ]

    p = pred.reshape(B, N).astype(np.float64)
    probs = np.clip(np.stack([p, 1.0 - p], -1), CLIP, 1.0)  # [B,N,2]
    negU = np.log(probs)
    Q0 = probs / probs.sum(-1, keepdims=True)

    full = np.empty((N, FRESH_COLS), np.float32)
    full[:, 0:3] = cb[0]
    full[:, 3:6] = cb[1]
    full[:, 6] = -q2b[0]
    full[:, 7] = -q2b[1]
    full[:, 8:10] = Q0[0]
    full[:, 10:12] = Q0[1]
    full[:, 12:14] = negU[0]
    full[:, 14:16] = negU[1]
    per_core = [
        np.ascontiguousarray(full[c * CHUNK : (c + 1) * CHUNK])
        for c in range(N_CORES)
    ]
    return per_core, Q0


# ---------------- PJRT runner (cached across calls) ----------------

class _Runner:
    def __init__(self, iters: int):
        import jax
        from jax.sharding import Mesh, PartitionSpec, NamedSharding
        from jax.experimental.shard_map import shard_map
        from concourse import bass2jax

        self.jax = jax
        nc = build_program(iters)
        bass2jax.install_neuronx_cc_hook()

        partition_name = (
            nc.partition_id_tensor.name if nc.partition_id_tensor else None
        )
        in_names, out_names, out_avals = [], [], []
        zero_outs = []
        for alloc in nc.m.functions[0].allocations:
            if not isinstance(alloc, mybir.MemoryLocationSet):
                continue
            name = alloc.memorylocations[0].name
            if alloc.kind == "ExternalInput":
                if name != partition_name:
                    in_names.append(name)
            elif alloc.kind == "ExternalOutput":
                shape = tuple(alloc.tensor_shape)
                dtype = mybir.dt.np(alloc.dtype)
                out_names.append(name)
                out_avals.append(jax.core.ShapedArray(shape, dtype))
                zero_outs.append(np.zeros(shape, dtype))
        self.in_names = in_names
        self.out_names = out_names
        self.out_avals = out_avals
        all_in_names = list(in_names) + list(out_names)
        if partition_name is not None:
            all_in_names.append(partition_name)

        def _body(*args):
            operands = list(args)
            if partition_name is not None:
                operands.append(bass2jax.partition_id_tensor())
            outs = bass2jax._bass_exec_p.bind(
                *operands,
                out_avals=tuple(out_avals),
                in_names=tuple(all_in_names),
                out_names=tuple(out_names),
                lowering_input_output_aliases=(),
                sim_require_finite=True,
                sim_require_nnan=True,
                nc=nc,
            )
            return tuple(outs)

        devices = jax.devices()[:N_CORES]
        mesh = Mesh(np.asarray(devices), ("core",))
        n_in = len(in_names) + len(zero_outs)
        sharded = jax.jit(
            shard_map(
                _body,
                mesh=mesh,
                in_specs=(PartitionSpec("core"),) * n_in,
                out_specs=(PartitionSpec("core"),) * len(out_names),
                check_rep=False,
            ),
            keep_unused=True,
        )
        self.sharded = sharded
        self._body = _body
        self.mesh = mesh
        sh = NamedSharding(mesh, PartitionSpec("core"))
        self.sh = sh

        smaps = static_inputs()
        self.static = {
            nm: jax.device_put(
                np.concatenate([smaps[c][nm] for c in range(N_CORES)], 0), sh
            )
            for nm in smaps[0]
        }
        self.zeros = [
            jax.device_put(
                np.zeros((N_CORES * z.shape[0], *z.shape[1:]), z.dtype), sh
            )
            for z in zero_outs
        ]
        jax.block_until_ready(list(self.static.values()))
        jax.block_until_ready(self.zeros)

    def args_for(self, fresh_per_core, device_put_fresh=False):
        args = []
        for nm in self.in_names:
            if nm == "fresh":
                a = np.concatenate(fresh_per_core, 0)
                if device_put_fresh:
                    a = self.jax.device_put(a, self.sh)
                args.append(a)
            else:
                args.append(self.static[nm])
        return args

    def __call__(self, fresh_per_core):
        outs = self.sharded(*self.args_for(fresh_per_core), *self.zeros)
        q = np.asarray(outs[self.out_names.index("qout")])
        return q.reshape(N_CORES * CHUNK, 2)


def get_runner(iters: int) -> "_Runner":
    if iters not in _RUNNER_CACHE:
        _RUNNER_CACHE[iters] = _Runner(iters)
    return _RUNNER_CACHE[iters]


def kernel(img, pred, iters):
    img = np.asarray(img, dtype=np.float32)
    pred = np.asarray(pred, dtype=np.float32)
    iters = int(np.asarray(iters))

    fresh, Q0 = fresh_inputs(img, pred)
    if iters <= 0:
        return np.ascontiguousarray(
            Q0[..., 0].astype(np.float32).reshape(B, 1, H, W)
        )

    runner = get_runner(iters)
    prob0 = runner(fresh)  # [N, 2], columns = image index
    out = np.stack(
        [prob0[:, 0].reshape(1, H, W), prob0[:, 1].reshape(1, H, W)], axis=0
    ).astype(np.float32)
    return out

